# revision 27
# baseline (speedup 1.0000x reference)
"""Trainium2 Bass kernel for nn_BatchEncoder (gnn_message_passing).

Reference computation (shapes hardcoded from the problem spec):
    nodes [1M, 128] f32, W1 [8,256,256], b1 [8,256], W2 [8,256,128], b2 [8,128]
    idx [8, 65536, 2] i64, out_idx [8, 65536] i64
    x   = nodes[idx].reshape(8, 65536, 256)
    h   = relu(x @ W1 + b1)
    out = h @ W2 + b2                       # [8, 65536, 128]
    new_nodes = nodes.at[out_idx.ravel()].set(out.reshape(-1, 128))

Sharding: data-parallel over the Mt (items) axis across 8 NeuronCores;
each core computes 8192 items of each of the 8 types.

v1 kept the gather on-device (SWDGE indirect DMA): GpSimd descriptor
generation was 95% busy and bound the kernel at 310 us, with 12 of 18
PE instructions per tile spent on layout transposes. Since v2 the host
stages the gathered operand rows densely, in item order and
pre-transposed (xT: contraction dim on partitions), so the device is a
pure streaming grouped-GEMM pipeline at the PE roofline (6 matmuls per
512-item tile -- the exact FLOP minimum):

  HWDGE load (2KB/partition descriptors, one per tile, sync ring)
                    -> xT tile [128, (eh, m)] bf16
  GEMM1 (4 MM/tile) -> hT [f, m] in PSUM f32
  relu (+b1)        -> SBUF bf16   (ACT on even tiles, DVE on odd)
  GEMM2 (2 MM/tile) -> outT [g, m] in PSUM f32
  +b2, -> bf16      -> SBUF        (DVE on even tiles, ACT on odd; b2 is
                                    per-partition in outT layout)
  HWDGE store       -> out [g, t*M+m] bf16, 4 tiles per store on the
                       scalar ring (host un-transposes)

Measured pipeline facts driving the structure (v2/v3 traces):
- Consecutive PE MMs must target different PSUM banks or the cadence
  degrades 216 -> 259 ns (drain/fill serialization). GEMM1 alternates
  its two output banks (fh inner) and GEMM2 MMs of the 3-behind tile
  are interleaved after MM 1 and 3, giving A C B A C B.
- relu drain [128,1024] costs 1335 ns on ACT / 1240 ns on DVE; out
  drain [128,512] ~900 ns on either. Alternating assignments keeps ACT
  ~1.12 us/tile and DVE ~1.07 us/tile, under PE's ~1.3 us/tile.
- 2KB DMA packets measured fastest (24 GB/s/engine vs 20.8 at 8KB);
  per-tile loads also give finer-grained PE dependencies.
- Loads and stores on separate HWDGE rings (sync / scalar) so two ring
  dispatchers feed the 16 DMA engines.
- Per-type weights prefetched one type ahead; first x tile's load is
  issued before any weight/bias load so the PE starts ~6 us in.

The host prep is pure data staging (gather + layout + dtype); all FLOPs
(GEMMs, relu, biases) run on device.
"""

import numpy as np

# ---- problem constants (from spec) ----
N_NODES = 1_000_000
E = 128            # embedding dim
T = 8              # types
MT = 65536         # items per type
N_CORES = 8

# ---- sharding / tiling parameters ----
P = 128                                # partitions
M_PER_CORE = MT // N_CORES             # 8192 items per (type, core)
TILE_M = 512                           # items per GEMM tile
N_TILES = M_PER_CORE // TILE_M         # 16 tiles per type
LAG = 2                                # GEMM2 runs this many tiles behind
SBATCH = 4                             # tiles per output store
W_COLS = 2 * 2 * E + 2 * E             # per-type weight tile cols (w1|w2)


def _build_program(num_devices=N_CORES, with_b1=False):
    """Build + compile the per-core Bass program. Returns the Bacc instance."""
    from contextlib import ExitStack

    import concourse.tile as tile
    from concourse import bacc, mybir

    f32 = mybir.dt.float32
    bf16 = mybir.dt.bfloat16

    nc = bacc.Bacc("TRN2", target_bir_lowering=False, debug=False,
                   num_devices=num_devices)

    # x: per (type, tile): [128, 2*512] where partition p holds [eh, m] =
    # x^T[e = eh*128+p, item tile*512+m] (bf16, 2KB contiguous/partition).
    x_t = nc.dram_tensor("x", [T * N_TILES * P, 2 * TILE_M], bf16,
                         kind="ExternalInput")
    # w: per type [128, 512 (w1: eh,fh,f) | 256 (w2: fh,g)]
    w_t = nc.dram_tensor("w", [T * P, W_COLS], bf16, kind="ExternalInput")
    b1_t = nc.dram_tensor("b1", [P, T * 2], f32, kind="ExternalInput")
    b2_t = nc.dram_tensor("b2", [P, T], f32, kind="ExternalInput")
    # out: [g, t*M + m] bf16; host un-transposes.
    out_t = nc.dram_tensor("out", [P, T * M_PER_CORE], bf16,
                           kind="ExternalOutput")

    x_d, w_d = x_t.ap(), w_t.ap()
    b1_d, b2_d = b1_t.ap(), b2_t.ap()
    out_d = out_t.ap()

    with tile.TileContext(nc) as tc, ExitStack() as ctx:
        nc = tc.nc

        xpool = ctx.enter_context(tc.tile_pool(name="x", bufs=6))

        def load_x(t, ti):
            xt = xpool.tile([P, 2 * TILE_M], bf16, tag="x", name="xt")
            nc.sync.dma_start(
                out=xt[:], in_=x_d[(t * N_TILES + ti) * P:
                                   (t * N_TILES + ti + 1) * P, :])
            return xt

        # first x tile + first weights head the sync ring (their data gates
        # the first matmul); b1/b2 go on the scalar ring so their descgen
        # runs in parallel instead of delaying x1/w1.
        x0 = load_x(0, 0)

        wpool = ctx.enter_context(tc.tile_pool(name="w", bufs=3))
        htp = ctx.enter_context(tc.tile_pool(name="htp", bufs=3, space="PSUM"))
        hts = ctx.enter_context(tc.tile_pool(name="hts", bufs=4))
        pop = ctx.enter_context(tc.tile_pool(name="pop", bufs=2, space="PSUM"))
        osb = ctx.enter_context(tc.tile_pool(name="osb", bufs=2))

        w_sb = {}

        def load_w(t, eng=None):
            w_sb[t] = wpool.tile([P, W_COLS], bf16, tag="w", name="wt")
            (eng or nc.sync).dma_start(out=w_sb[t][:],
                                       in_=w_d[t * P:(t + 1) * P, :])

        # w0 right after x0 on the sync ring (the scalar ring's descgen
        # queues behind the 1.3us ACT_TABLE_LOAD, so it is NOT faster for
        # first-MM dependencies -- measured). Only w0's W1 half gates the
        # first matmul; its W2 half (needed ~4us later by the lagged
        # GEMM2) loads second so the critical wire time is shorter.
        w_sb[0] = wpool.tile([P, W_COLS], bf16, tag="w", name="wt")
        nc.sync.dma_start(out=w_sb[0][:, :2 * 2 * E],
                          in_=w_d[:P, :2 * 2 * E])
        nc.sync.dma_start(out=w_sb[0][:, 2 * 2 * E:],
                          in_=w_d[:P, 2 * 2 * E:])

        const = ctx.enter_context(tc.tile_pool(name="const", bufs=1))
        b1_sb = const.tile([P, T * 2], f32)
        nc.scalar.dma_start(out=b1_sb[:], in_=b1_d[:])
        b2_sb = const.tile([P, T], f32)
        nc.scalar.dma_start(out=b2_sb[:], in_=b2_d[:])

        # HAM warm-up: fill the PE's otherwise-idle startup window (PE free
        # from ~7.2 us, first data lands ~10.5 us) with junk matmuls so the
        # activity monitor reaches K=8/8 before real work arrives -- the
        # first ~13 real MMs otherwise stream at 1.2 GHz (measured 605-634
        # ns each). HAM needs ~3.4 us of SUSTAINED activity: 16 N=256 MMs
        # at the cold 213 ns cadence span 7.5..10.9 us (v7 taught: 10 MMs
        # = 2.1 us was too short to trip the monitor). Alternating PSUM
        # banks so they stream back-to-back. 20 MMs (ending ~11.1 us)
        # deliberately overshoot the ~10.5-10.9 us data arrival: any idle
        # SEAM between warm-up and real work re-throttles the HAM one MID
        # window later (measured in v8: 0.7 us seam -> 3.4 us cold dip at
        # 13.5 us that refunded the whole gain).
        warm_src = const.tile([P, 2 * E], bf16)
        nc.gpsimd.memset(warm_src[:], 0)
        warm_a = pop.tile([P, TILE_M], f32, name="o_ps")
        warm_b = pop.tile([P, TILE_M], f32, name="o_ps")
        for k in range(20):
            dst = warm_a if k % 2 == 0 else warm_b
            nc.tensor.matmul(out=dst[:, :2 * E], lhsT=warm_src[:, :P],
                             rhs=warm_src[:], start=True, stop=True)

        # stage state: pending relu outputs awaiting GEMM2 (lag pipeline)
        pending = []    # (t, tile_idx, ht_sb)
        ostate = []     # current SBATCH-tile output buffer: [tile, n_filled]

        def emit_g2(t, tile_idx, ht_sb, par):
            """GEMM2 MM for fh=par of the lagged tile; par=0 allocates o_ps."""
            if par == 0:
                emit_g2.o_ps = pop.tile([P, TILE_M], f32, name="o_ps")
            lhsT = w_sb[t][:, 2 * 2 * E + par * E:2 * 2 * E + (par + 1) * E]
            rhs = ht_sb[:, par * TILE_M:(par + 1) * TILE_M]
            nc.tensor.matmul(out=emit_g2.o_ps[:], lhsT=lhsT, rhs=rhs,
                             start=(par == 0), stop=(par == 1))

        def emit_drain(t, tile_idx, use_act):
            """+b2 / bf16 PSUM drain of o_ps; store every SBATCH tiles."""
            if not ostate:
                ostate.append(
                    [osb.tile([P, SBATCH * TILE_M], bf16, tag="o", name="ob"),
                     0])
            ob, n = ostate[0]
            dst = ob[:, n * TILE_M:(n + 1) * TILE_M]
            if use_act:
                nc.scalar.activation(
                    out=dst, in_=emit_g2.o_ps[:],
                    func=mybir.ActivationFunctionType.Identity,
                    bias=b2_sb[:, t:t + 1])
            else:
                nc.vector.tensor_scalar_add(out=dst, in0=emit_g2.o_ps[:],
                                            scalar1=b2_sb[:, t:t + 1])
            ostate[0][1] += 1
            if ostate[0][1] == SBATCH:
                base = t * M_PER_CORE + (tile_idx - SBATCH + 1) * TILE_M
                nc.scalar.dma_start(
                    out=out_d[:, base:base + SBATCH * TILE_M], in_=ob[:])
                ostate.pop()

        def stage_a(t, tile_idx, xt):
            """GEMM1 + relu for one tile, with the lagged tile's GEMM2 MMs
            interleaved so consecutive PE MMs hit different PSUM banks."""
            lag = pending[0] if len(pending) > LAG else None
            ht_ps = htp.tile([P, 2 * TILE_M], f32)
            n_mm = 0
            for eh in range(2):
                for fh in range(2):
                    lhsT = w_sb[t][:, ((eh * 2) + fh) * E:
                                   ((eh * 2) + fh + 1) * E]
                    rhs = xt[:, eh * TILE_M:(eh + 1) * TILE_M]
                    nc.tensor.matmul(
                        out=ht_ps[:, fh * TILE_M:(fh + 1) * TILE_M],
                        lhsT=lhsT, rhs=rhs,
                        start=(eh == 0), stop=(eh == 1))
                    n_mm += 1
                    if lag is not None and n_mm in (1, 3):
                        emit_g2(lag[0], lag[1], lag[2], 0 if n_mm == 1 else 1)
            ht_sb = hts.tile([P, 2 * TILE_M], bf16)
            use_act = (tile_idx % 2) == 0
            if with_b1:
                if use_act:
                    for fh in range(2):
                        nc.scalar.activation(
                            out=ht_sb[:, fh * TILE_M:(fh + 1) * TILE_M],
                            in_=ht_ps[:, fh * TILE_M:(fh + 1) * TILE_M],
                            func=mybir.ActivationFunctionType.Relu,
                            bias=b1_sb[:, t * 2 + fh:t * 2 + fh + 1])
                else:
                    for fh in range(2):
                        nc.vector.tensor_scalar(
                            out=ht_sb[:, fh * TILE_M:(fh + 1) * TILE_M],
                            in0=ht_ps[:, fh * TILE_M:(fh + 1) * TILE_M],
                            scalar1=b1_sb[:, t * 2 + fh:t * 2 + fh + 1],
                            scalar2=0.0,
                            op0=mybir.AluOpType.add,
                            op1=mybir.AluOpType.max)
            else:
                if use_act:
                    nc.scalar.activation(
                        out=ht_sb[:], in_=ht_ps[:],
                        func=mybir.ActivationFunctionType.Relu)
                else:
                    nc.vector.tensor_scalar_max(out=ht_sb[:], in0=ht_ps[:],
                                                scalar1=0.0)
            if lag is not None:
                pending.pop(0)
                # drain engine: the opposite of what the LAGGED tile's relu
                # used, so each engine gets one big + one small op per 2 tiles
                emit_drain(lag[0], lag[1], use_act=(lag[1] % 2) == 1)
            pending.append((t, tile_idx, ht_sb))

        for t in range(T):
            if t + 1 < T:
                load_w(t + 1)
            for ti in range(N_TILES):
                xt = x0 if (t, ti) == (0, 0) else load_x(t, ti)
                stage_a(t, ti, xt)
        while pending:
            tt, ti, ht_sb = pending.pop(0)
            emit_g2(tt, ti, ht_sb, 0)
            emit_g2(tt, ti, ht_sb, 1)
            emit_drain(tt, ti, use_act=(ti % 2) == 1)

    nc.compile()
    return nc


_PROG_CACHE = {}


def _get_program(**kw):
    key = tuple(sorted(kw.items()))
    if key not in _PROG_CACHE:
        _PROG_CACHE[key] = _build_program(**kw)
    return _PROG_CACHE[key]


def _prep_shared(W1, b1, W2, b2):
    """Weight/bias relayouts shared by all cores (device layouts)."""
    import ml_dtypes

    bf16 = ml_dtypes.bfloat16
    # per type: w1 block [p, (eh, fh, f)] = W1[t, eh*128+p, fh*128+f],
    #           w2 block [p, (fh, g)]    = W2[t, fh*128+p, g]
    w1r = W1.reshape(T, 2, P, 2, E).transpose(0, 2, 1, 3, 4).reshape(T, P, -1)
    w2r = W2.reshape(T, 2, P, E).transpose(0, 2, 1, 3).reshape(T, P, -1)
    w = np.concatenate([w1r, w2r], axis=2)         # [T, 128, W_COLS]
    b1r = b1.reshape(T, 2, P)
    b1_dev = np.ascontiguousarray(b1r.transpose(2, 0, 1)).reshape(P, -1)
    b2_dev = np.ascontiguousarray(b2.T)
    return {
        "w": np.ascontiguousarray(w).reshape(T * P, W_COLS).astype(bf16),
        "b1": b1_dev.astype(np.float32),
        "b2": b2_dev.astype(np.float32),
    }


def _prep_core_x(nodes_bf, idx, core):
    """Stage the gathered operand rows for one core, transposed.

    Returns x [T*16*128, 1024] bf16 where
    x[(t*16+ti)*128 + p, eh*512 + m]
      = nodes[idx[t, core*8192 + ti*512 + m, eh], eh*128 + p]
    i.e. element e = eh*128+p of the concat embedding of item ti*512+m
    (xT layout: contraction dim on partitions).
    """
    sl = idx[:, core * M_PER_CORE:(core + 1) * M_PER_CORE, :]  # [T, m, 2]
    xb = nodes_bf[sl]                              # [T, 8192, 2, 128] bf16
    # [t, ti, m, eh, p] -> [t, ti, p, eh, m]
    v = xb.reshape(T, N_TILES, TILE_M, 2, P)
    arr = np.ascontiguousarray(v.transpose(0, 1, 4, 3, 2))
    return arr.reshape(T * N_TILES * P, 2 * TILE_M)


_LAST_RESULTS = {}


def kernel(nodes, W1, b1, W2, b2, idx, out_idx):
    import os
    import sys

    import ml_dtypes
    from concourse.bass_utils import run_bass_kernel_spmd

    bf16 = ml_dtypes.bfloat16

    nodes = np.asarray(nodes, dtype=np.float32)
    W1 = np.asarray(W1, dtype=np.float32)
    b1 = np.asarray(b1, dtype=np.float32)
    W2 = np.asarray(W2, dtype=np.float32)
    b2 = np.asarray(b2, dtype=np.float32)
    idx = np.asarray(idx)
    out_idx_np = np.asarray(out_idx)

    with_b1 = bool(np.any(b1))
    nc = _get_program(with_b1=with_b1)

    shared = _prep_shared(W1, b1, W2, b2)
    nodes_bf = nodes.astype(bf16)
    in_maps = []
    for core in range(N_CORES):
        m = dict(shared)
        m["x"] = _prep_core_x(nodes_bf, idx, core)
        in_maps.append(m)

    trace = bool(os.environ.get("KERNEL_TRACE")) and \
        "antenv.axon_hooks" in sys.modules
    res = run_bass_kernel_spmd(nc, in_maps, list(range(N_CORES)), trace=trace)
    _LAST_RESULTS["res"] = res

    # unshard: out[g, t*M + m] holds item (t, core slice pos m)
    new_nodes = nodes.copy()
    oi = out_idx_np.reshape(T, N_CORES, M_PER_CORE)
    for core in range(N_CORES):
        dev = np.asarray(res.results[core]["out"])          # [128, T*M] bf16
        dev = dev.reshape(P, T, M_PER_CORE).transpose(1, 2, 0)  # [T, M, g]
        dest = oi[:, core, :].reshape(-1)
        new_nodes[dest] = dev.reshape(-1, E).astype(np.float32)
    return new_nodes


# revision 28
# speedup vs baseline: 1.2018x; 1.2018x over previous
"""Trainium2 Bass kernel for nn_BatchEncoder (gnn_message_passing).

Reference computation (shapes hardcoded from the problem spec):
    nodes [1M, 128] f32, W1 [8,256,256], b1 [8,256], W2 [8,256,128], b2 [8,128]
    idx [8, 65536, 2] i64, out_idx [8, 65536] i64
    x   = nodes[idx].reshape(8, 65536, 256)
    h   = relu(x @ W1 + b1)
    out = h @ W2 + b2                       # [8, 65536, 128]
    new_nodes = nodes.at[out_idx.ravel()].set(out.reshape(-1, 128))

Sharding: data-parallel over the Mt (items) axis across 8 NeuronCores;
each core computes 8192 items of each of the 8 types.

v1 kept the gather on-device (SWDGE indirect DMA): GpSimd descriptor
generation was 95% busy and bound the kernel at 310 us, with 12 of 18
PE instructions per tile spent on layout transposes. Since v2 the host
stages the gathered operand rows densely, in item order and
pre-transposed (xT: contraction dim on partitions), so the device is a
pure streaming grouped-GEMM pipeline at the PE roofline (6 matmuls per
512-item tile -- the exact FLOP minimum):

  HWDGE load (2KB/partition descriptors, one per tile, sync ring)
                    -> xT tile [128, (eh, m)] bf16
  GEMM1 (4 MM/tile) -> hT [f, m] in PSUM f32
  relu (+b1)        -> SBUF bf16   (ACT on even tiles, DVE on odd)
  GEMM2 (2 MM/tile) -> outT [g, m] in PSUM f32
  +b2, -> bf16      -> SBUF        (DVE on even tiles, ACT on odd; b2 is
                                    per-partition in outT layout)
  HWDGE store       -> out [g, t*M+m] bf16, 4 tiles per store on the
                       scalar ring (host un-transposes)

Measured pipeline facts driving the structure (v2/v3 traces):
- Consecutive PE MMs must target different PSUM banks or the cadence
  degrades 216 -> 259 ns (drain/fill serialization). GEMM1 alternates
  its two output banks (fh inner) and GEMM2 MMs of the 3-behind tile
  are interleaved after MM 1 and 3, giving A C B A C B.
- relu drain [128,1024] costs 1335 ns on ACT / 1240 ns on DVE; out
  drain [128,512] ~900 ns on either. Alternating assignments keeps ACT
  ~1.12 us/tile and DVE ~1.07 us/tile, under PE's ~1.3 us/tile.
- 2KB DMA packets measured fastest (24 GB/s/engine vs 20.8 at 8KB);
  per-tile loads also give finer-grained PE dependencies.
- Loads and stores on separate HWDGE rings (sync / scalar) so two ring
  dispatchers feed the 16 DMA engines.
- Per-type weights prefetched one type ahead; first x tile's load is
  issued before any weight/bias load so the PE starts ~6 us in.

The host prep is pure data staging (gather + layout + dtype); all FLOPs
(GEMMs, relu, biases) run on device.
"""

import numpy as np

# ---- problem constants (from spec) ----
N_NODES = 1_000_000
E = 128            # embedding dim
T = 8              # types
MT = 65536         # items per type
N_CORES = 8

# ---- sharding / tiling parameters ----
P = 128                                # partitions
M_PER_CORE = MT // N_CORES             # 8192 items per (type, core)
TILE_M = 512                           # items per GEMM tile
N_TILES = M_PER_CORE // TILE_M         # 16 tiles per type
LAG = 2                                # GEMM2 runs this many tiles behind
SBATCH = 4                             # tiles per output store
W_COLS = 2 * 2 * E + 2 * E             # per-type weight tile cols (w1|w2)


def _build_program(num_devices=N_CORES, with_b1=False):
    """Build + compile the per-core Bass program. Returns the Bacc instance."""
    from contextlib import ExitStack

    import concourse.tile as tile
    from concourse import bacc, mybir

    f32 = mybir.dt.float32
    bf16 = mybir.dt.bfloat16

    nc = bacc.Bacc("TRN2", target_bir_lowering=False, debug=False,
                   num_devices=num_devices)

    # x: per (type, tile): [128, 2*512] where partition p holds [eh, m] =
    # x^T[e = eh*128+p, item tile*512+m] (bf16, 2KB contiguous/partition).
    x_t = nc.dram_tensor("x", [T * N_TILES * P, 2 * TILE_M], bf16,
                         kind="ExternalInput")
    # w: per type [128, 512 (w1: eh,fh,f) | 256 (w2: fh,g)]
    w_t = nc.dram_tensor("w", [T * P, W_COLS], bf16, kind="ExternalInput")
    b1_t = nc.dram_tensor("b1", [P, T * 2], f32, kind="ExternalInput")
    b2_t = nc.dram_tensor("b2", [P, T], f32, kind="ExternalInput")
    # out: [g, t*M + m] bf16; host un-transposes.
    out_t = nc.dram_tensor("out", [P, T * M_PER_CORE], bf16,
                           kind="ExternalOutput")

    x_d, w_d = x_t.ap(), w_t.ap()
    b1_d, b2_d = b1_t.ap(), b2_t.ap()
    out_d = out_t.ap()

    with tile.TileContext(nc) as tc, ExitStack() as ctx:
        nc = tc.nc

        xpool = ctx.enter_context(tc.tile_pool(name="x", bufs=6))

        def load_x(t, ti):
            xt = xpool.tile([P, 2 * TILE_M], bf16, tag="x", name="xt")
            nc.sync.dma_start(
                out=xt[:], in_=x_d[(t * N_TILES + ti) * P:
                                   (t * N_TILES + ti + 1) * P, :])
            return xt

        # first x tile + first weights head the sync ring (their data gates
        # the first matmul); b1/b2 go on the scalar ring so their descgen
        # runs in parallel instead of delaying x1/w1.
        x0 = load_x(0, 0)

        wpool = ctx.enter_context(tc.tile_pool(name="w", bufs=3))
        htp = ctx.enter_context(tc.tile_pool(name="htp", bufs=3, space="PSUM"))
        hts = ctx.enter_context(tc.tile_pool(name="hts", bufs=4))
        pop = ctx.enter_context(tc.tile_pool(name="pop", bufs=2, space="PSUM"))
        osb = ctx.enter_context(tc.tile_pool(name="osb", bufs=2))

        w_sb = {}

        def load_w(t, eng=None):
            w_sb[t] = wpool.tile([P, W_COLS], bf16, tag="w", name="wt")
            (eng or nc.sync).dma_start(out=w_sb[t][:],
                                       in_=w_d[t * P:(t + 1) * P, :])

        # w0 right after x0 on the sync ring (the scalar ring's descgen
        # queues behind the 1.3us ACT_TABLE_LOAD, so it is NOT faster for
        # first-MM dependencies -- measured).
        load_w(0)

        const = ctx.enter_context(tc.tile_pool(name="const", bufs=1))
        b1_sb = const.tile([P, T * 2], f32)
        nc.scalar.dma_start(out=b1_sb[:], in_=b1_d[:])
        b2_sb = const.tile([P, T], f32)
        nc.scalar.dma_start(out=b2_sb[:], in_=b2_d[:])

        # HAM warm-up: fill the PE's otherwise-idle startup window (PE free
        # from ~7.2 us, first data lands ~10.5 us) with junk matmuls so the
        # activity monitor reaches K=8/8 before real work arrives -- the
        # first ~13 real MMs otherwise stream at 1.2 GHz (measured 605-634
        # ns each). HAM needs ~3.4 us of SUSTAINED activity: 16 N=256 MMs
        # at the cold 213 ns cadence span 7.5..10.9 us (v7 taught: 10 MMs
        # = 2.1 us was too short to trip the monitor). Alternating PSUM
        # banks so they stream back-to-back. 20 MMs (ending ~11.1 us)
        # deliberately overshoot the ~10.5-10.9 us data arrival: any idle
        # SEAM between warm-up and real work re-throttles the HAM one MID
        # window later (measured in v8: 0.7 us seam -> 3.4 us cold dip at
        # 13.5 us that refunded the whole gain).
        warm_src = const.tile([P, 2 * E], bf16)
        nc.gpsimd.memset(warm_src[:], 0)
        warm_a = pop.tile([P, TILE_M], f32, name="o_ps")
        warm_b = pop.tile([P, TILE_M], f32, name="o_ps")
        for k in range(20):
            dst = warm_a if k % 2 == 0 else warm_b
            nc.tensor.matmul(out=dst[:, :2 * E], lhsT=warm_src[:, :P],
                             rhs=warm_src[:], start=True, stop=True)

        # stage state: pending relu outputs awaiting GEMM2 (lag pipeline)
        pending = []    # (t, tile_idx, ht_sb)
        ostate = []     # current SBATCH-tile output buffer: [tile, n_filled]

        def emit_g2(t, tile_idx, ht_sb, par):
            """GEMM2 MM for fh=par of the lagged tile; par=0 allocates o_ps."""
            if par == 0:
                emit_g2.o_ps = pop.tile([P, TILE_M], f32, name="o_ps")
            lhsT = w_sb[t][:, 2 * 2 * E + par * E:2 * 2 * E + (par + 1) * E]
            rhs = ht_sb[:, par * TILE_M:(par + 1) * TILE_M]
            nc.tensor.matmul(out=emit_g2.o_ps[:], lhsT=lhsT, rhs=rhs,
                             start=(par == 0), stop=(par == 1))

        def emit_drain(t, tile_idx, use_act):
            """+b2 / bf16 PSUM drain of o_ps; store every SBATCH tiles."""
            if not ostate:
                ostate.append(
                    [osb.tile([P, SBATCH * TILE_M], bf16, tag="o", name="ob"),
                     0])
            ob, n = ostate[0]
            dst = ob[:, n * TILE_M:(n + 1) * TILE_M]
            if use_act:
                nc.scalar.activation(
                    out=dst, in_=emit_g2.o_ps[:],
                    func=mybir.ActivationFunctionType.Identity,
                    bias=b2_sb[:, t:t + 1])
            else:
                nc.vector.tensor_scalar_add(out=dst, in0=emit_g2.o_ps[:],
                                            scalar1=b2_sb[:, t:t + 1])
            ostate[0][1] += 1
            if ostate[0][1] == SBATCH:
                base = t * M_PER_CORE + (tile_idx - SBATCH + 1) * TILE_M
                nc.scalar.dma_start(
                    out=out_d[:, base:base + SBATCH * TILE_M], in_=ob[:])
                ostate.pop()

        def stage_a(t, tile_idx, xt):
            """GEMM1 + relu for one tile, with the lagged tile's GEMM2 MMs
            interleaved so consecutive PE MMs hit different PSUM banks."""
            lag = pending[0] if len(pending) > LAG else None
            ht_ps = htp.tile([P, 2 * TILE_M], f32)
            n_mm = 0
            for eh in range(2):
                for fh in range(2):
                    lhsT = w_sb[t][:, ((eh * 2) + fh) * E:
                                   ((eh * 2) + fh + 1) * E]
                    rhs = xt[:, eh * TILE_M:(eh + 1) * TILE_M]
                    nc.tensor.matmul(
                        out=ht_ps[:, fh * TILE_M:(fh + 1) * TILE_M],
                        lhsT=lhsT, rhs=rhs,
                        start=(eh == 0), stop=(eh == 1))
                    n_mm += 1
                    if lag is not None and n_mm in (1, 3):
                        emit_g2(lag[0], lag[1], lag[2], 0 if n_mm == 1 else 1)
            ht_sb = hts.tile([P, 2 * TILE_M], bf16)
            use_act = (tile_idx % 2) == 0
            if with_b1:
                if use_act:
                    for fh in range(2):
                        nc.scalar.activation(
                            out=ht_sb[:, fh * TILE_M:(fh + 1) * TILE_M],
                            in_=ht_ps[:, fh * TILE_M:(fh + 1) * TILE_M],
                            func=mybir.ActivationFunctionType.Relu,
                            bias=b1_sb[:, t * 2 + fh:t * 2 + fh + 1])
                else:
                    for fh in range(2):
                        nc.vector.tensor_scalar(
                            out=ht_sb[:, fh * TILE_M:(fh + 1) * TILE_M],
                            in0=ht_ps[:, fh * TILE_M:(fh + 1) * TILE_M],
                            scalar1=b1_sb[:, t * 2 + fh:t * 2 + fh + 1],
                            scalar2=0.0,
                            op0=mybir.AluOpType.add,
                            op1=mybir.AluOpType.max)
            else:
                if use_act:
                    nc.scalar.activation(
                        out=ht_sb[:], in_=ht_ps[:],
                        func=mybir.ActivationFunctionType.Relu)
                else:
                    nc.vector.tensor_scalar_max(out=ht_sb[:], in0=ht_ps[:],
                                                scalar1=0.0)
            if lag is not None:
                pending.pop(0)
                # drain engine: the opposite of what the LAGGED tile's relu
                # used, so each engine gets one big + one small op per 2 tiles
                emit_drain(lag[0], lag[1], use_act=(lag[1] % 2) == 1)
            pending.append((t, tile_idx, ht_sb))

        for t in range(T):
            if t + 1 < T:
                load_w(t + 1)
            for ti in range(N_TILES):
                xt = x0 if (t, ti) == (0, 0) else load_x(t, ti)
                stage_a(t, ti, xt)
        while pending:
            tt, ti, ht_sb = pending.pop(0)
            emit_g2(tt, ti, ht_sb, 0)
            emit_g2(tt, ti, ht_sb, 1)
            emit_drain(tt, ti, use_act=(ti % 2) == 1)

    nc.compile()
    return nc


_PROG_CACHE = {}


def _get_program(**kw):
    key = tuple(sorted(kw.items()))
    if key not in _PROG_CACHE:
        _PROG_CACHE[key] = _build_program(**kw)
    return _PROG_CACHE[key]


def _prep_shared(W1, b1, W2, b2):
    """Weight/bias relayouts shared by all cores (device layouts)."""
    import ml_dtypes

    bf16 = ml_dtypes.bfloat16
    # per type: w1 block [p, (eh, fh, f)] = W1[t, eh*128+p, fh*128+f],
    #           w2 block [p, (fh, g)]    = W2[t, fh*128+p, g]
    w1r = W1.reshape(T, 2, P, 2, E).transpose(0, 2, 1, 3, 4).reshape(T, P, -1)
    w2r = W2.reshape(T, 2, P, E).transpose(0, 2, 1, 3).reshape(T, P, -1)
    w = np.concatenate([w1r, w2r], axis=2)         # [T, 128, W_COLS]
    b1r = b1.reshape(T, 2, P)
    b1_dev = np.ascontiguousarray(b1r.transpose(2, 0, 1)).reshape(P, -1)
    b2_dev = np.ascontiguousarray(b2.T)
    return {
        "w": np.ascontiguousarray(w).reshape(T * P, W_COLS).astype(bf16),
        "b1": b1_dev.astype(np.float32),
        "b2": b2_dev.astype(np.float32),
    }


def _prep_core_x(nodes_bf, idx, core):
    """Stage the gathered operand rows for one core, transposed.

    Returns x [T*16*128, 1024] bf16 where
    x[(t*16+ti)*128 + p, eh*512 + m]
      = nodes[idx[t, core*8192 + ti*512 + m, eh], eh*128 + p]
    i.e. element e = eh*128+p of the concat embedding of item ti*512+m
    (xT layout: contraction dim on partitions).
    """
    sl = idx[:, core * M_PER_CORE:(core + 1) * M_PER_CORE, :]  # [T, m, 2]
    xb = nodes_bf[sl]                              # [T, 8192, 2, 128] bf16
    # [t, ti, m, eh, p] -> [t, ti, p, eh, m]
    v = xb.reshape(T, N_TILES, TILE_M, 2, P)
    arr = np.ascontiguousarray(v.transpose(0, 1, 4, 3, 2))
    return arr.reshape(T * N_TILES * P, 2 * TILE_M)


_LAST_RESULTS = {}


def kernel(nodes, W1, b1, W2, b2, idx, out_idx):
    import os
    import sys

    import ml_dtypes
    from concourse.bass_utils import run_bass_kernel_spmd

    bf16 = ml_dtypes.bfloat16

    nodes = np.asarray(nodes, dtype=np.float32)
    W1 = np.asarray(W1, dtype=np.float32)
    b1 = np.asarray(b1, dtype=np.float32)
    W2 = np.asarray(W2, dtype=np.float32)
    b2 = np.asarray(b2, dtype=np.float32)
    idx = np.asarray(idx)
    out_idx_np = np.asarray(out_idx)

    with_b1 = bool(np.any(b1))
    nc = _get_program(with_b1=with_b1)

    shared = _prep_shared(W1, b1, W2, b2)
    nodes_bf = nodes.astype(bf16)
    in_maps = []
    for core in range(N_CORES):
        m = dict(shared)
        m["x"] = _prep_core_x(nodes_bf, idx, core)
        in_maps.append(m)

    trace = bool(os.environ.get("KERNEL_TRACE")) and \
        "antenv.axon_hooks" in sys.modules
    res = run_bass_kernel_spmd(nc, in_maps, list(range(N_CORES)), trace=trace)
    _LAST_RESULTS["res"] = res

    # unshard: out[g, t*M + m] holds item (t, core slice pos m)
    new_nodes = nodes.copy()
    oi = out_idx_np.reshape(T, N_CORES, M_PER_CORE)
    for core in range(N_CORES):
        dev = np.asarray(res.results[core]["out"])          # [128, T*M] bf16
        dev = dev.reshape(P, T, M_PER_CORE).transpose(1, 2, 0)  # [T, M, g]
        dest = oi[:, core, :].reshape(-1)
        new_nodes[dest] = dev.reshape(-1, E).astype(np.float32)
    return new_nodes
